# revision 18
# baseline (speedup 1.0000x reference)
"""Trainium2 Bass kernel for nn_Ir_Consistency_Loss (gnn_message_passing).

loss = mean_e (1 - re[src_e].re[dst_e]) * ||ir_h[src_e] - ir_h[dst_e]||^2

One-sided-gather design, edge-parallel across 8 NeuronCores.

The kernel's bottleneck is the Q7/Pool dma_gather descriptor path
(~8 ns per gathered row, engine-serial), so only the DST side is
gathered per edge; the SRC side is replicated on the PE:

  - Node features G = [re|ir] cast to bf16, split into half tables GA/GB
    (+zero pad rows) so dst-gather local row ids fit dma_gather's int16.
  - Edges bucketed by (src-half, dst-half), sharded across cores, then
    sorted by src. A group = 128 consecutive edges whose srcs lie in one
    128-node block; a tile = 32 groups = 4096 edges.
  - SRC side: the host pre-extracts each group's 128-row src block into a
    streamed tensor wtab [T*32*128, 256] and builds a one-hot
    [128 nodes x 128 edges] per group; a PE matmul (onehot^T @ block)
    replicates src rows into PSUM edge-major tiles. No per-edge gather.
  - DST side: one 4096-idx dma_gather per tile (PREPARE_ONLY + trigger,
    512B bf16 rows). Slot map j -> (j%128, j//128) makes group g line up
    exactly with v[:, g, :].
  - Pad edges have an all-zero one-hot column (u=0) and dst=zero row
    (v=0), so their contribution is exactly (1-0)*0 = 0 in any bucket.
  - DVE/ACT per 8-group PSUM batch: prod/reduce -> agree, diff, ACT
    square, reduce -> sqsum, then (agree-1)*sqsum accumulated into
    per-tile partials (negated loss).
  - Host: loss = -(sum of partials) / E.
"""

import numpy as np
import ml_dtypes

import concourse.bacc as bacc
import concourse.bass as bass
import concourse.mybir as mybir
import concourse.tile as tile
from concourse.bass_utils import run_bass_kernel_spmd

N_NODES = 50000
HALF = 25000
D = 128
N_CORES = 8
P = 128
GRP = 32                   # groups (of 128 edges) per tile
TILE_E = P * GRP           # 4096 edges per tile
IDX_COLS = TILE_E // 16    # int16 idx columns (wrap-16 layout)
PAD_ROW = HALF             # local id of an all-zero row in each table
TBL_ROWS = HALF + P        # half-table rows (zero padded; last block fits)

IDX_BUFS = 3
NB = 4                     # PSUM batches per tile
GPB = GRP // NB            # groups per batch (8)

_cache = {}


def _build_program(tiles_per_bucket):
    key = tuple(tiles_per_bucket)
    if key in _cache:
        return _cache[key]
    T = sum(tiles_per_bucket)
    nc = bacc.Bacc("TRN2", target_bir_lowering=False, debug=False,
                   num_devices=N_CORES)
    bf16 = mybir.dt.bfloat16
    fp32 = mybir.dt.float32
    ga = nc.dram_tensor("ga", [TBL_ROWS, 2 * D], bf16, kind="ExternalInput")
    gb = nc.dram_tensor("gb", [TBL_ROWS, 2 * D], bf16, kind="ExternalInput")
    didx = nc.dram_tensor("didx", [T * P, IDX_COLS], mybir.dt.int16,
                          kind="ExternalInput")
    oneh = nc.dram_tensor("oneh", [T * P, GRP * P], bf16,
                          kind="ExternalInput")
    wtab = nc.dram_tensor("wtab", [T * GRP * P, 2 * D], bf16,
                          kind="ExternalInput")
    out = nc.dram_tensor("partial", [P, 1], fp32, kind="ExternalOutput")

    Alu = mybir.AluOpType
    X = mybir.AxisListType.X
    Sq = mybir.ActivationFunctionType.Square
    dtab = [ga, gb, ga, gb]   # dst table by bucket

    with tile.TileContext(nc) as tc:
        gsems = [nc.alloc_semaphore(f"g{i}") for i in range(T)]
        with (
            tc.tile_pool(name="idx", bufs=IDX_BUFS) as ipool,
            tc.tile_pool(name="gath", bufs=3) as gpool,
            tc.tile_pool(name="win", bufs=2) as wpool,
            tc.tile_pool(name="oh", bufs=2) as opool,
            tc.tile_pool(name="ps", bufs=2, space="PSUM") as pspool,
            tc.tile_pool(name="scr", bufs=2) as spool,
            tc.tile_pool(name="stats", bufs=1) as stpool,
        ):
            partials = stpool.tile([P, T * NB], fp32, tag="partials")
            t = 0
            for b in range(4):
                for _ in range(tiles_per_bucket[b]):
                    ei = ipool.tile([P, IDX_COLS], mybir.dt.int16, tag="ei")
                    if t >= IDX_BUFS:
                        nc.gpsimd.wait_ge(gsems[t - IDX_BUFS], 16)
                    nc.gpsimd.dma_start(out=ei[:],
                                        in_=didx[t * P:(t + 1) * P, :])

                    v = gpool.tile([P, GRP, 2 * D], bf16, tag="v")
                    nc.gpsimd.dma_gather(v[:], dtab[b][:], ei[:], TILE_E,
                                         TILE_E, 2 * D, single_packet=False,
                                         prepare_only=True, sem=gsems[t])
                    nc.gpsimd.trigger_dma(count=None)

                    # group g's src block rows: wtab[(t*GRP+g)*P + k] ->
                    # xw[k, g, :]
                    xw = wpool.tile([P, GRP, 2 * D], bf16, tag="xw")
                    base = t * GRP * P
                    win_ap = bass.AP(
                        tensor=wtab[:].tensor,
                        offset=base * 2 * D,
                        ap=[[2 * D, P], [P * 2 * D, GRP], [1, 2 * D]])
                    nc.sync.dma_start(out=xw[:], in_=win_ap)

                    oh = opool.tile([P, GRP, P], bf16, tag="oh")
                    nc.sync.dma_start(out=oh[:],
                                      in_=oneh[t * P:(t + 1) * P, :])

                    first = True
                    for nb in range(NB):
                        ups = pspool.tile([P, GPB, 2 * D], fp32, tag="ups")
                        for gg in range(GPB):
                            g = nb * GPB + gg
                            nc.tensor.matmul(ups[:, gg, :], oh[:, g, :],
                                             xw[:, g, :],
                                             start=True, stop=True)
                        vs = v[:, nb * GPB:(nb + 1) * GPB, :]

                        prod = spool.tile([P, GPB, D], bf16, tag="prod")
                        agree = spool.tile([P, GPB], fp32, tag="agree")
                        diff = spool.tile([P, GPB, D], bf16, tag="diff")
                        sq = spool.tile([P, GPB, D], bf16, tag="sq")
                        sqsum = spool.tile([P, GPB], fp32, tag="sqsum")
                        junk = spool.tile([P, GPB], fp32, tag="junk")

                        if first:
                            # manual RAW sync with the prepared gather's
                            # DMA completion (16 engines x 1 prep).
                            nc.vector.wait_ge(gsems[t], 16)
                            first = False
                        nc.vector.tensor_tensor(out=prod[:],
                                                in0=ups[:, :, 0:D],
                                                in1=vs[:, :, 0:D],
                                                op=Alu.mult)
                        nc.vector.tensor_reduce(out=agree[:], in_=prod[:],
                                                axis=X, op=Alu.add)
                        nc.vector.tensor_tensor(out=diff[:],
                                                in0=ups[:, :, D:2 * D],
                                                in1=vs[:, :, D:2 * D],
                                                op=Alu.subtract)
                        nc.scalar.activation(out=sq[:], in_=diff[:], func=Sq)
                        nc.vector.tensor_reduce(out=sqsum[:], in_=sq[:],
                                                axis=X, op=Alu.add)
                        k = t * NB + nb
                        nc.vector.scalar_tensor_tensor(
                            out=junk[:], in0=agree[:], scalar=1.0,
                            in1=sqsum[:], op0=Alu.subtract, op1=Alu.mult,
                            accum_out=partials[:, k:k + 1])
                    t += 1

            total = stpool.tile([P, 1], fp32, tag="total")
            nc.vector.tensor_reduce(out=total[:], in_=partials[:], axis=X,
                                    op=Alu.add)
            nc.sync.dma_start(out=out[:], in_=total[:])
    nc.compile()
    _cache[key] = nc
    return nc


def _wrap_idx(flat_idx):
    """[n_tiles, TILE_E] local ids -> [n_tiles*P, IDX_COLS] int16 blocks.
    Logical j -> [j % 16, j // 16], replicated on all 8 16-row groups."""
    nt = flat_idx.shape[0]
    j = np.arange(TILE_E)
    w = np.zeros((nt, 16, IDX_COLS), np.int16)
    w[:, j % 16, j // 16] = flat_idx.astype(np.int16)
    return np.ascontiguousarray(np.tile(w, (1, 8, 1))).reshape(nt * P, IDX_COLS)


def _layout_bucket(sb, db):
    """Sort one core-bucket's edges by src and lay out groups/tiles.

    Returns (n_tiles, dst_slots [n_tiles*TILE_E], block_of_group
    [n_tiles*GRP] (-1 = pad group), k_local, g_global, m) where the last
    three map each real edge to its one-hot coordinates."""
    order = np.argsort(sb, kind="stable")
    sb = sb[order]
    db = db[order]
    blk = sb // P
    ub, start, cnt = np.unique(blk, return_index=True, return_counts=True)
    gpb = -(-cnt // P)                       # groups per block
    gbase = np.concatenate([[0], np.cumsum(gpb)])
    n_groups = int(gbase[-1])
    n_tiles = max(1, -(-n_groups // GRP))
    # per-edge coordinates
    r = np.arange(len(sb)) - np.repeat(start, cnt)   # rank within block
    g_global = np.repeat(gbase[:-1], cnt) + r // P
    m = r % P
    k_local = sb - blk * P
    # block id per group (pad groups -> -1)
    block_of_group = np.full(n_tiles * GRP, -1, np.int64)
    block_of_group[:n_groups] = np.repeat(ub, gpb)
    # dst slot array (pads -> PAD_ROW)
    dst_slots = np.full(n_tiles * TILE_E, PAD_ROW, np.int64)
    dst_slots[g_global * P + m] = db
    return n_tiles, dst_slots, block_of_group, k_local, g_global, m


def kernel(re_, ir_h, src, dst):
    re_ = np.asarray(re_, dtype=np.float32)
    ir_h = np.asarray(ir_h, dtype=np.float32)
    g = np.concatenate([re_, ir_h], axis=1).astype(ml_dtypes.bfloat16)
    ga = np.zeros((TBL_ROWS, 2 * D), ml_dtypes.bfloat16)
    gb = np.zeros((TBL_ROWS, 2 * D), ml_dtypes.bfloat16)
    ga[:HALF] = g[:HALF]
    gb[:HALF] = g[HALF:]
    stab_np = [ga, ga, gb, gb]

    s = np.asarray(src).astype(np.int64)
    d = np.asarray(dst).astype(np.int64)
    e_total = s.shape[0]
    bucket = (s >= HALF) * 2 + (d >= HALF)

    # per (core, bucket) layouts
    layouts = [[None] * 4 for _ in range(N_CORES)]
    for b in range(4):
        mask = bucket == b
        sb_all = s[mask] - (HALF if b >= 2 else 0)
        db_all = d[mask] - (HALF if b % 2 == 1 else 0)
        n = sb_all.shape[0]
        for c in range(N_CORES):
            lo = (n * c) // N_CORES
            hi = (n * (c + 1)) // N_CORES
            layouts[c][b] = _layout_bucket(sb_all[lo:hi], db_all[lo:hi])

    # shared per-bucket tile counts (max over cores)
    tiles_per_bucket = tuple(
        max(layouts[c][b][0] for c in range(N_CORES)) for b in range(4))
    T = sum(tiles_per_bucket)

    in_maps = []
    for c in range(N_CORES):
        dst_flat = np.full((T, TILE_E), PAD_ROW, np.int64)
        oneh = np.zeros((T * P, GRP * P), ml_dtypes.bfloat16)
        blocks = np.full(T * GRP, -1, np.int64)
        t0 = 0
        for b in range(4):
            nt, dst_slots, bog, k_local, g_global, m = layouts[c][b]
            dst_flat[t0:t0 + nt] = dst_slots.reshape(nt, TILE_E)
            blocks[t0 * GRP:t0 * GRP + nt * GRP] = bog
            # one-hot entries for real edges
            tl = g_global // GRP + t0
            gi = g_global % GRP
            oneh[tl * P + k_local, gi * P + m] = 1
            t0 += tiles_per_bucket[b]
        # window table: rows for (tile, group) = the group's 128-block
        wt_nodes = np.zeros((T * GRP, P), np.int64)
        real = blocks >= 0
        wt_nodes[real] = blocks[real, None] * P + np.arange(P)[None, :]
        wtab = np.zeros((T * GRP * P, 2 * D), ml_dtypes.bfloat16)
        # fill per bucket from its src table (pad groups stay zero)
        t0 = 0
        for b in range(4):
            nt = tiles_per_bucket[b]
            sl = slice(t0 * GRP, (t0 + nt) * GRP)
            rb = real[sl]
            rows = wt_nodes[sl][rb]
            block_rows = stab_np[b][np.minimum(rows, TBL_ROWS - 1)]
            wfull = wtab[t0 * GRP * P:(t0 + nt) * GRP * P].reshape(
                nt * GRP, P, 2 * D)
            wfull[rb] = block_rows
            t0 += nt
        in_maps.append({"ga": ga, "gb": gb,
                        "didx": _wrap_idx(dst_flat),
                        "oneh": np.ascontiguousarray(oneh),
                        "wtab": np.ascontiguousarray(wtab)})

    nc = _build_program(tiles_per_bucket)
    res = run_bass_kernel_spmd(nc, in_maps, core_ids=list(range(N_CORES)))
    tot = 0.0
    for r in res.results:
        tot += float(r["partial"].sum(dtype=np.float64))
    return np.float32(-tot / e_total)


# revision 25
# speedup vs baseline: 1.9968x; 1.9968x over previous
"""Trainium2 Bass kernel for nn_Ir_Consistency_Loss (gnn_message_passing).

loss = mean_e (1 - re[src_e].re[dst_e]) * ||ir_h[src_e] - ir_h[dst_e]||^2

One-sided-gather design, edge-parallel across 8 NeuronCores.

The kernel's bottleneck is the Q7/Pool dma_gather descriptor path
(~8 ns per gathered row, engine-serial), so only the DST side is
gathered per edge; the SRC side is replicated on the PE:

  - Node features G = [re|ir] cast to bf16, split into half tables GA/GB
    (+zero pad rows) so dst-gather local row ids fit dma_gather's int16.
  - Edges bucketed by (src-half, dst-half), sharded across cores, then
    sorted by src. A group = 128 consecutive edges whose srcs lie in one
    128-node block; a tile = 32 groups = 4096 edges.
  - SRC side: the host pre-extracts each group's 128-row src block into a
    streamed tensor wtab [T*32*128, 256] and builds a one-hot
    [128 nodes x 128 edges] per group; a PE matmul (onehot^T @ block)
    replicates src rows into PSUM edge-major tiles. No per-edge gather.
  - DST side: one 4096-idx dma_gather per tile (PREPARE_ONLY + trigger,
    512B bf16 rows). Slot map j -> (j%128, j//128) makes group g line up
    exactly with v[:, g, :].
  - Pad edges have an all-zero one-hot column (u=0) and dst=zero row
    (v=0), so their contribution is exactly (1-0)*0 = 0 in any bucket.
  - DVE/ACT per 8-group PSUM batch: prod/reduce -> agree, diff, ACT
    square, reduce -> sqsum, then (agree-1)*sqsum accumulated into
    per-tile partials (negated loss).
  - Host: loss = -(sum of partials) / E.
"""

import numpy as np
import ml_dtypes

import concourse.bacc as bacc
import concourse.bass as bass
import concourse.mybir as mybir
import concourse.tile as tile
from concourse.bass_utils import run_bass_kernel_spmd

N_NODES = 50000
HALF = 25000
D = 128
N_CORES = 8
P = 128
GRP = 32                   # groups (of 128 edges) per tile
TILE_E = P * GRP           # 4096 edges per tile
IDX_COLS = TILE_E // 16    # int16 idx columns (wrap-16 layout)
PAD_ROW = HALF             # local id of an all-zero row in each table
TBL_ROWS = HALF + P        # half-table rows (zero padded; last block fits)

IDX_BUFS = 3
NB = 4                     # PSUM batches per tile
GPB = GRP // NB            # groups per batch (8)

_cache = {}


def _build_program(tiles_per_bucket):
    key = tuple(tiles_per_bucket)
    if key in _cache:
        return _cache[key]
    T = sum(tiles_per_bucket)
    nc = bacc.Bacc("TRN2", target_bir_lowering=False, debug=False,
                   num_devices=N_CORES)
    bf16 = mybir.dt.bfloat16
    fp32 = mybir.dt.float32
    ga = nc.dram_tensor("ga", [TBL_ROWS, 2 * D], bf16, kind="ExternalInput")
    gb = nc.dram_tensor("gb", [TBL_ROWS, 2 * D], bf16, kind="ExternalInput")
    didx = nc.dram_tensor("didx", [T * P, IDX_COLS], mybir.dt.int16,
                          kind="ExternalInput")
    oneh = nc.dram_tensor("oneh", [T * P, GRP * P], bf16,
                          kind="ExternalInput")
    wtab = nc.dram_tensor("wtab", [T * GRP * P, 2 * D], bf16,
                          kind="ExternalInput")
    out = nc.dram_tensor("partial", [P, 1], fp32, kind="ExternalOutput")

    Alu = mybir.AluOpType
    X = mybir.AxisListType.X
    Sq = mybir.ActivationFunctionType.Square
    dtab = [ga, gb, ga, gb]   # dst table by bucket

    with tile.TileContext(nc) as tc:
        with (
            tc.tile_pool(name="idx", bufs=IDX_BUFS) as ipool,
            tc.tile_pool(name="gath", bufs=3) as gpool,
            tc.tile_pool(name="win", bufs=2) as wpool,
            tc.tile_pool(name="oh", bufs=2) as opool,
            tc.tile_pool(name="ps", bufs=2, space="PSUM") as pspool,
            tc.tile_pool(name="scr", bufs=2) as spool,
            tc.tile_pool(name="stats", bufs=1) as stpool,
        ):
            partials = stpool.tile([P, T * NB], fp32, tag="partials")
            t = 0
            for b in range(4):
                for _ in range(tiles_per_bucket[b]):
                    ei = ipool.tile([P, IDX_COLS], mybir.dt.int16, tag="ei")
                    nc.sync.dma_start(out=ei[:],
                                      in_=didx[t * P:(t + 1) * P, :])

                    v = gpool.tile([P, GRP, 2 * D], bf16, tag="v")
                    nc.gpsimd.dma_gather(v[:], dtab[b][:], ei[:], TILE_E,
                                         TILE_E, 2 * D, single_packet=False)

                    # group g's src block rows: wtab[(t*GRP+g)*P + k] ->
                    # xw[k, g, :]
                    xw = wpool.tile([P, GRP, 2 * D], bf16, tag="xw")
                    base = t * GRP * P
                    win_ap = bass.AP(
                        tensor=wtab[:].tensor,
                        offset=base * 2 * D,
                        ap=[[2 * D, P], [P * 2 * D, GRP], [1, 2 * D]])
                    nc.sync.dma_start(out=xw[:], in_=win_ap)

                    oh = opool.tile([P, GRP, P], bf16, tag="oh")
                    nc.sync.dma_start(out=oh[:],
                                      in_=oneh[t * P:(t + 1) * P, :])

                    for nb in range(NB):
                        ups = pspool.tile([P, GPB, 2 * D], fp32, tag="ups")
                        for gg in range(GPB):
                            g = nb * GPB + gg
                            nc.tensor.matmul(ups[:, gg, :], oh[:, g, :],
                                             xw[:, g, :],
                                             start=True, stop=True)
                        vs = v[:, nb * GPB:(nb + 1) * GPB, :]

                        prod = spool.tile([P, GPB, D], bf16, tag="prod")
                        agree = spool.tile([P, GPB], fp32, tag="agree")
                        diff = spool.tile([P, GPB, D], bf16, tag="diff")
                        sq = spool.tile([P, GPB, D], bf16, tag="sq")
                        sqsum = spool.tile([P, GPB], fp32, tag="sqsum")
                        junk = spool.tile([P, GPB], fp32, tag="junk")

                        nc.vector.tensor_tensor(out=prod[:],
                                                in0=ups[:, :, 0:D],
                                                in1=vs[:, :, 0:D],
                                                op=Alu.mult)
                        nc.vector.tensor_reduce(out=agree[:], in_=prod[:],
                                                axis=X, op=Alu.add)
                        nc.vector.tensor_tensor(out=diff[:],
                                                in0=ups[:, :, D:2 * D],
                                                in1=vs[:, :, D:2 * D],
                                                op=Alu.subtract)
                        nc.scalar.activation(out=sq[:], in_=diff[:], func=Sq)
                        nc.vector.tensor_reduce(out=sqsum[:], in_=sq[:],
                                                axis=X, op=Alu.add)
                        k = t * NB + nb
                        nc.vector.scalar_tensor_tensor(
                            out=junk[:], in0=agree[:], scalar=1.0,
                            in1=sqsum[:], op0=Alu.subtract, op1=Alu.mult,
                            accum_out=partials[:, k:k + 1])
                    t += 1

            total = stpool.tile([P, 1], fp32, tag="total")
            nc.vector.tensor_reduce(out=total[:], in_=partials[:], axis=X,
                                    op=Alu.add)
            nc.sync.dma_start(out=out[:], in_=total[:])
    nc.compile()
    _cache[key] = nc
    return nc


def _wrap_idx(flat_idx):
    """[n_tiles, TILE_E] local ids -> [n_tiles*P, IDX_COLS] int16 blocks.
    Logical j -> [j % 16, j // 16], replicated on all 8 16-row groups."""
    nt = flat_idx.shape[0]
    j = np.arange(TILE_E)
    w = np.zeros((nt, 16, IDX_COLS), np.int16)
    w[:, j % 16, j // 16] = flat_idx.astype(np.int16)
    return np.ascontiguousarray(np.tile(w, (1, 8, 1))).reshape(nt * P, IDX_COLS)


def _layout_bucket(sb, db):
    """Sort one core-bucket's edges by src and lay out greedy groups of
    128 consecutive edges; each group's src block starts at its first
    edge's src (any alignment). 128 consecutive src-sorted edges span far
    fewer than 128 distinct nodes here, so k_local always fits.

    Returns (n_tiles, dst_slots [n_tiles*TILE_E], base_of_group
    [n_tiles*GRP] (-1 = pad group), k_local, g_global, m)."""
    order = np.argsort(sb, kind="stable")
    sb = sb[order]
    db = db[order]
    ne = len(sb)
    starts, bases = [], []
    i = 0
    while i < ne:
        starts.append(i)
        bases.append(int(sb[i]))
        i = min(i + P, int(np.searchsorted(sb, sb[i] + P, side="left")))
    n_groups = max(1, len(starts))
    n_tiles = -(-n_groups // GRP)
    base_of_group = np.full(n_tiles * GRP, -1, np.int64)
    g_global = np.zeros(ne, np.int64)
    m = np.zeros(ne, np.int64)
    if ne:
        base_of_group[:len(starts)] = bases
        ends = starts[1:] + [ne]
        cnt = np.array(ends) - np.array(starts)
        g_global = np.repeat(np.arange(len(starts)), cnt)
        m = np.arange(ne) - np.repeat(np.array(starts), cnt)
    k_local = sb - base_of_group[g_global] if ne else np.zeros(0, np.int64)
    dst_slots = np.full(n_tiles * TILE_E, PAD_ROW, np.int64)
    dst_slots[g_global * P + m] = db
    return n_tiles, dst_slots, base_of_group, k_local, g_global, m


def kernel(re_, ir_h, src, dst):
    re_ = np.asarray(re_, dtype=np.float32)
    ir_h = np.asarray(ir_h, dtype=np.float32)
    g = np.concatenate([re_, ir_h], axis=1).astype(ml_dtypes.bfloat16)
    ga = np.zeros((TBL_ROWS, 2 * D), ml_dtypes.bfloat16)
    gb = np.zeros((TBL_ROWS, 2 * D), ml_dtypes.bfloat16)
    ga[:HALF] = g[:HALF]
    gb[:HALF] = g[HALF:]
    stab_np = [ga, ga, gb, gb]

    s = np.asarray(src).astype(np.int64)
    d = np.asarray(dst).astype(np.int64)
    e_total = s.shape[0]
    bucket = (s >= HALF) * 2 + (d >= HALF)

    # per (core, bucket) layouts
    layouts = [[None] * 4 for _ in range(N_CORES)]
    for b in range(4):
        mask = bucket == b
        sb_all = s[mask] - (HALF if b >= 2 else 0)
        db_all = d[mask] - (HALF if b % 2 == 1 else 0)
        n = sb_all.shape[0]
        for c in range(N_CORES):
            lo = (n * c) // N_CORES
            hi = (n * (c + 1)) // N_CORES
            layouts[c][b] = _layout_bucket(sb_all[lo:hi], db_all[lo:hi])

    # shared per-bucket tile counts (max over cores)
    tiles_per_bucket = tuple(
        max(layouts[c][b][0] for c in range(N_CORES)) for b in range(4))
    T = sum(tiles_per_bucket)

    in_maps = []
    for c in range(N_CORES):
        dst_flat = np.full((T, TILE_E), PAD_ROW, np.int64)
        oneh = np.zeros((T * P, GRP * P), ml_dtypes.bfloat16)
        blocks = np.full(T * GRP, -1, np.int64)
        t0 = 0
        for b in range(4):
            nt, dst_slots, bog, k_local, g_global, m = layouts[c][b]
            dst_flat[t0:t0 + nt] = dst_slots.reshape(nt, TILE_E)
            blocks[t0 * GRP:t0 * GRP + nt * GRP] = bog
            # one-hot entries for real edges
            tl = g_global // GRP + t0
            gi = g_global % GRP
            oneh[tl * P + k_local, gi * P + m] = 1
            t0 += tiles_per_bucket[b]
        # window table: rows for (tile, group) = the group's 128-row block
        wt_nodes = np.zeros((T * GRP, P), np.int64)
        real = blocks >= 0
        wt_nodes[real] = blocks[real, None] + np.arange(P)[None, :]
        wtab = np.zeros((T * GRP * P, 2 * D), ml_dtypes.bfloat16)
        # fill per bucket from its src table (pad groups stay zero)
        t0 = 0
        for b in range(4):
            nt = tiles_per_bucket[b]
            sl = slice(t0 * GRP, (t0 + nt) * GRP)
            rb = real[sl]
            rows = wt_nodes[sl][rb]
            block_rows = stab_np[b][np.minimum(rows, TBL_ROWS - 1)]
            wfull = wtab[t0 * GRP * P:(t0 + nt) * GRP * P].reshape(
                nt * GRP, P, 2 * D)
            wfull[rb] = block_rows
            t0 += nt
        in_maps.append({"ga": ga, "gb": gb,
                        "didx": _wrap_idx(dst_flat),
                        "oneh": np.ascontiguousarray(oneh),
                        "wtab": np.ascontiguousarray(wtab)})

    nc = _build_program(tiles_per_bucket)
    res = run_bass_kernel_spmd(nc, in_maps, core_ids=list(range(N_CORES)))
    tot = 0.0
    for r in res.results:
        tot += float(r["partial"].sum(dtype=np.float64))
    return np.float32(-tot / e_total)


# revision 26
# speedup vs baseline: 2.2205x; 1.1120x over previous
"""Trainium2 Bass kernel for nn_Ir_Consistency_Loss (gnn_message_passing).

loss = mean_e (1 - re[src_e].re[dst_e]) * ||ir_h[src_e] - ir_h[dst_e]||^2

One-sided-gather design, edge-parallel across 8 NeuronCores.

The kernel's bottleneck is the Q7/Pool dma_gather descriptor path
(~8 ns per gathered row, engine-serial), so only the DST side is
gathered per edge; the SRC side is replicated on the PE:

  - Node features G = [re|ir] cast to bf16, split into half tables GA/GB
    (+zero pad rows) so dst-gather local row ids fit dma_gather's int16.
  - Edges bucketed by (src-half, dst-half), sharded across cores, then
    sorted by src. A group = 128 consecutive edges whose srcs lie in one
    128-node block; a tile = 32 groups = 4096 edges.
  - SRC side: the host pre-extracts each group's 128-row src block into a
    streamed tensor wtab [T*32*128, 256] and builds a one-hot
    [128 nodes x 128 edges] per group; a PE matmul (onehot^T @ block)
    replicates src rows into PSUM edge-major tiles. No per-edge gather.
  - DST side: one 4096-idx dma_gather per tile (PREPARE_ONLY + trigger,
    512B bf16 rows). Slot map j -> (j%128, j//128) makes group g line up
    exactly with v[:, g, :].
  - Pad edges have an all-zero one-hot column (u=0) and dst=zero row
    (v=0), so their contribution is exactly (1-0)*0 = 0 in any bucket.
  - DVE/ACT per 8-group PSUM batch: prod/reduce -> agree, diff, ACT
    square, reduce -> sqsum, then (agree-1)*sqsum accumulated into
    per-tile partials (negated loss).
  - Host: loss = -(sum of partials) / E.
"""

import numpy as np
import ml_dtypes

import concourse.bacc as bacc
import concourse.bass as bass
import concourse.mybir as mybir
import concourse.tile as tile
from concourse.bass_utils import run_bass_kernel_spmd

N_NODES = 50000
HALF = 25000
D = 128
N_CORES = 8
P = 128
GRP = 32                   # groups (of 128 edges) per tile
TILE_E = P * GRP           # 4096 edges per tile
IDX_COLS = TILE_E // 16    # int16 idx columns (wrap-16 layout)
PAD_ROW = HALF             # local id of an all-zero row in each table
TBL_ROWS = HALF + P        # half-table rows (zero padded; last block fits)

IDX_BUFS = 3
NB = 4                     # PSUM batches per tile
GPB = GRP // NB            # groups per batch (8)

_cache = {}


def _build_program(tiles_per_bucket):
    key = tuple(tiles_per_bucket)
    if key in _cache:
        return _cache[key]
    T = sum(tiles_per_bucket)
    nc = bacc.Bacc("TRN2", target_bir_lowering=False, debug=False,
                   num_devices=N_CORES)
    bf16 = mybir.dt.bfloat16
    fp32 = mybir.dt.float32
    ga = nc.dram_tensor("ga", [TBL_ROWS, 2 * D], bf16, kind="ExternalInput")
    gb = nc.dram_tensor("gb", [TBL_ROWS, 2 * D], bf16, kind="ExternalInput")
    didx = nc.dram_tensor("didx", [T * P, IDX_COLS], mybir.dt.int16,
                          kind="ExternalInput")
    oneh = nc.dram_tensor("oneh", [T * P, GRP * P], bf16,
                          kind="ExternalInput")
    wtab = nc.dram_tensor("wtab", [T * GRP * P, 2 * D], bf16,
                          kind="ExternalInput")
    out = nc.dram_tensor("partial", [P, 1], fp32, kind="ExternalOutput")

    Alu = mybir.AluOpType
    X = mybir.AxisListType.X
    Sq = mybir.ActivationFunctionType.Square
    dtab = [ga, gb, ga, gb]   # dst table by bucket

    with tile.TileContext(nc) as tc:
        with (
            tc.tile_pool(name="idx", bufs=4) as ipool,
            tc.tile_pool(name="gath", bufs=4) as gpool,
            tc.tile_pool(name="win", bufs=3) as wpool,
            tc.tile_pool(name="oh", bufs=3) as opool,
            tc.tile_pool(name="ps", bufs=2, space="PSUM") as pspool,
            tc.tile_pool(name="scr", bufs=3) as spool,
            tc.tile_pool(name="stats", bufs=1) as stpool,
        ):
            partials = stpool.tile([P, T * NB], fp32, tag="partials")
            t = 0
            for b in range(4):
                for _ in range(tiles_per_bucket[b]):
                    ei = ipool.tile([P, IDX_COLS], mybir.dt.int16, tag="ei")
                    nc.sync.dma_start(out=ei[:],
                                      in_=didx[t * P:(t + 1) * P, :])

                    v = gpool.tile([P, GRP, 2 * D], bf16, tag="v")
                    nc.gpsimd.dma_gather(v[:], dtab[b][:], ei[:], TILE_E,
                                         TILE_E, 2 * D, single_packet=False)

                    # group g's src block rows: wtab[(t*GRP+g)*P + k] ->
                    # xw[k, g, :]
                    xw = wpool.tile([P, GRP, 2 * D], bf16, tag="xw")
                    base = t * GRP * P
                    win_ap = bass.AP(
                        tensor=wtab[:].tensor,
                        offset=base * 2 * D,
                        ap=[[2 * D, P], [P * 2 * D, GRP], [1, 2 * D]])
                    nc.sync.dma_start(out=xw[:], in_=win_ap)

                    oh = opool.tile([P, GRP, P], bf16, tag="oh")
                    nc.sync.dma_start(out=oh[:],
                                      in_=oneh[t * P:(t + 1) * P, :])

                    for nb in range(NB):
                        ups = pspool.tile([P, GPB, 2 * D], fp32, tag="ups")
                        for gg in range(GPB):
                            g = nb * GPB + gg
                            nc.tensor.matmul(ups[:, gg, :], oh[:, g, :],
                                             xw[:, g, :],
                                             start=True, stop=True)
                        vs = v[:, nb * GPB:(nb + 1) * GPB, :]

                        prod = spool.tile([P, GPB, D], bf16, tag="prod")
                        agree = spool.tile([P, GPB], fp32, tag="agree")
                        diff = spool.tile([P, GPB, D], bf16, tag="diff")
                        sq = spool.tile([P, GPB, D], bf16, tag="sq")
                        sqsum = spool.tile([P, GPB], fp32, tag="sqsum")
                        junk = spool.tile([P, GPB], fp32, tag="junk")

                        nc.vector.tensor_tensor(out=prod[:],
                                                in0=ups[:, :, 0:D],
                                                in1=vs[:, :, 0:D],
                                                op=Alu.mult)
                        nc.vector.tensor_reduce(out=agree[:], in_=prod[:],
                                                axis=X, op=Alu.add)
                        nc.vector.tensor_tensor(out=diff[:],
                                                in0=ups[:, :, D:2 * D],
                                                in1=vs[:, :, D:2 * D],
                                                op=Alu.subtract)
                        nc.scalar.activation(out=sq[:], in_=diff[:], func=Sq)
                        nc.vector.tensor_reduce(out=sqsum[:], in_=sq[:],
                                                axis=X, op=Alu.add)
                        k = t * NB + nb
                        nc.vector.scalar_tensor_tensor(
                            out=junk[:], in0=agree[:], scalar=1.0,
                            in1=sqsum[:], op0=Alu.subtract, op1=Alu.mult,
                            accum_out=partials[:, k:k + 1])
                    t += 1

            total = stpool.tile([P, 1], fp32, tag="total")
            nc.vector.tensor_reduce(out=total[:], in_=partials[:], axis=X,
                                    op=Alu.add)
            nc.sync.dma_start(out=out[:], in_=total[:])
    nc.compile()
    _cache[key] = nc
    return nc


def _wrap_idx(flat_idx):
    """[n_tiles, TILE_E] local ids -> [n_tiles*P, IDX_COLS] int16 blocks.
    Logical j -> [j % 16, j // 16], replicated on all 8 16-row groups."""
    nt = flat_idx.shape[0]
    j = np.arange(TILE_E)
    w = np.zeros((nt, 16, IDX_COLS), np.int16)
    w[:, j % 16, j // 16] = flat_idx.astype(np.int16)
    return np.ascontiguousarray(np.tile(w, (1, 8, 1))).reshape(nt * P, IDX_COLS)


def _layout_bucket(sb, db):
    """Sort one core-bucket's edges by src and lay out greedy groups of
    128 consecutive edges; each group's src block starts at its first
    edge's src (any alignment). 128 consecutive src-sorted edges span far
    fewer than 128 distinct nodes here, so k_local always fits.

    Returns (n_tiles, dst_slots [n_tiles*TILE_E], base_of_group
    [n_tiles*GRP] (-1 = pad group), k_local, g_global, m)."""
    order = np.argsort(sb, kind="stable")
    sb = sb[order]
    db = db[order]
    ne = len(sb)
    starts, bases = [], []
    i = 0
    while i < ne:
        starts.append(i)
        bases.append(int(sb[i]))
        i = min(i + P, int(np.searchsorted(sb, sb[i] + P, side="left")))
    n_groups = max(1, len(starts))
    n_tiles = -(-n_groups // GRP)
    base_of_group = np.full(n_tiles * GRP, -1, np.int64)
    g_global = np.zeros(ne, np.int64)
    m = np.zeros(ne, np.int64)
    if ne:
        base_of_group[:len(starts)] = bases
        ends = starts[1:] + [ne]
        cnt = np.array(ends) - np.array(starts)
        g_global = np.repeat(np.arange(len(starts)), cnt)
        m = np.arange(ne) - np.repeat(np.array(starts), cnt)
    k_local = sb - base_of_group[g_global] if ne else np.zeros(0, np.int64)
    dst_slots = np.full(n_tiles * TILE_E, PAD_ROW, np.int64)
    dst_slots[g_global * P + m] = db
    return n_tiles, dst_slots, base_of_group, k_local, g_global, m


def kernel(re_, ir_h, src, dst):
    re_ = np.asarray(re_, dtype=np.float32)
    ir_h = np.asarray(ir_h, dtype=np.float32)
    g = np.concatenate([re_, ir_h], axis=1).astype(ml_dtypes.bfloat16)
    ga = np.zeros((TBL_ROWS, 2 * D), ml_dtypes.bfloat16)
    gb = np.zeros((TBL_ROWS, 2 * D), ml_dtypes.bfloat16)
    ga[:HALF] = g[:HALF]
    gb[:HALF] = g[HALF:]
    stab_np = [ga, ga, gb, gb]

    s = np.asarray(src).astype(np.int64)
    d = np.asarray(dst).astype(np.int64)
    e_total = s.shape[0]
    bucket = (s >= HALF) * 2 + (d >= HALF)

    # per (core, bucket) layouts
    layouts = [[None] * 4 for _ in range(N_CORES)]
    for b in range(4):
        mask = bucket == b
        sb_all = s[mask] - (HALF if b >= 2 else 0)
        db_all = d[mask] - (HALF if b % 2 == 1 else 0)
        n = sb_all.shape[0]
        for c in range(N_CORES):
            lo = (n * c) // N_CORES
            hi = (n * (c + 1)) // N_CORES
            layouts[c][b] = _layout_bucket(sb_all[lo:hi], db_all[lo:hi])

    # shared per-bucket tile counts (max over cores)
    tiles_per_bucket = tuple(
        max(layouts[c][b][0] for c in range(N_CORES)) for b in range(4))
    T = sum(tiles_per_bucket)

    in_maps = []
    for c in range(N_CORES):
        dst_flat = np.full((T, TILE_E), PAD_ROW, np.int64)
        oneh = np.zeros((T * P, GRP * P), ml_dtypes.bfloat16)
        blocks = np.full(T * GRP, -1, np.int64)
        t0 = 0
        for b in range(4):
            nt, dst_slots, bog, k_local, g_global, m = layouts[c][b]
            dst_flat[t0:t0 + nt] = dst_slots.reshape(nt, TILE_E)
            blocks[t0 * GRP:t0 * GRP + nt * GRP] = bog
            # one-hot entries for real edges
            tl = g_global // GRP + t0
            gi = g_global % GRP
            oneh[tl * P + k_local, gi * P + m] = 1
            t0 += tiles_per_bucket[b]
        # window table: rows for (tile, group) = the group's 128-row block
        wt_nodes = np.zeros((T * GRP, P), np.int64)
        real = blocks >= 0
        wt_nodes[real] = blocks[real, None] + np.arange(P)[None, :]
        wtab = np.zeros((T * GRP * P, 2 * D), ml_dtypes.bfloat16)
        # fill per bucket from its src table (pad groups stay zero)
        t0 = 0
        for b in range(4):
            nt = tiles_per_bucket[b]
            sl = slice(t0 * GRP, (t0 + nt) * GRP)
            rb = real[sl]
            rows = wt_nodes[sl][rb]
            block_rows = stab_np[b][np.minimum(rows, TBL_ROWS - 1)]
            wfull = wtab[t0 * GRP * P:(t0 + nt) * GRP * P].reshape(
                nt * GRP, P, 2 * D)
            wfull[rb] = block_rows
            t0 += nt
        in_maps.append({"ga": ga, "gb": gb,
                        "didx": _wrap_idx(dst_flat),
                        "oneh": np.ascontiguousarray(oneh),
                        "wtab": np.ascontiguousarray(wtab)})

    nc = _build_program(tiles_per_bucket)
    res = run_bass_kernel_spmd(nc, in_maps, core_ids=list(range(N_CORES)))
    tot = 0.0
    for r in res.results:
        tot += float(r["partial"].sum(dtype=np.float64))
    return np.float32(-tot / e_total)


# revision 29
# speedup vs baseline: 2.3123x; 1.0413x over previous
"""Trainium2 Bass kernel for nn_Ir_Consistency_Loss (gnn_message_passing).

loss = mean_e (1 - re[src_e].re[dst_e]) * ||ir_h[src_e] - ir_h[dst_e]||^2

One-sided-gather design, edge-parallel across 8 NeuronCores.

The kernel's bottleneck is the Q7/Pool dma_gather descriptor path
(~8 ns per gathered row, engine-serial), so only the DST side is
gathered per edge; the SRC side is replicated on the PE:

  - Node features G = [re|ir] cast to bf16, split into half tables GA/GB
    (+zero pad rows) so dst-gather local row ids fit dma_gather's int16.
  - Edges bucketed by (src-half, dst-half), sharded across cores, then
    sorted by src. A group = 128 consecutive edges whose srcs lie in one
    128-node block; a tile = 32 groups = 4096 edges.
  - SRC side: the host pre-extracts each group's 128-row src block into a
    streamed tensor wtab [T*32*128, 256] and builds a one-hot
    [128 nodes x 128 edges] per group; a PE matmul (onehot^T @ block)
    replicates src rows into PSUM edge-major tiles. No per-edge gather.
  - DST side: one 4096-idx dma_gather per tile (PREPARE_ONLY + trigger,
    512B bf16 rows). Slot map j -> (j%128, j//128) makes group g line up
    exactly with v[:, g, :].
  - Pad edges have an all-zero one-hot column (u=0) and dst=zero row
    (v=0), so their contribution is exactly (1-0)*0 = 0 in any bucket.
  - DVE/ACT per 8-group PSUM batch: prod/reduce -> agree, diff, ACT
    square, reduce -> sqsum, then (agree-1)*sqsum accumulated into
    per-tile partials (negated loss).
  - Host: loss = -(sum of partials) / E.
"""

import numpy as np
import ml_dtypes

import concourse.bacc as bacc
import concourse.bass as bass
import concourse.mybir as mybir
import concourse.tile as tile
from concourse.bass_utils import run_bass_kernel_spmd

N_NODES = 50000
HALF = 25000
D = 128
N_CORES = 8
P = 128
GRP = 32                   # groups (of 128 edges) per tile
TILE_E = P * GRP           # 4096 edges per tile
IDX_COLS = TILE_E // 16    # int16 idx columns (wrap-16 layout)
PAD_ROW = HALF             # local id of an all-zero row in each table
TBL_ROWS = HALF + P        # half-table rows (zero padded; last block fits)

IDX_BUFS = 3
NB = 4                     # PSUM batches per tile
GPB = GRP // NB            # groups per batch (8)

_cache = {}


def _build_program(tiles_per_bucket):
    key = tuple(tiles_per_bucket)
    if key in _cache:
        return _cache[key]
    T = sum(tiles_per_bucket)
    nc = bacc.Bacc("TRN2", target_bir_lowering=False, debug=False,
                   num_devices=N_CORES)
    bf16 = mybir.dt.bfloat16
    fp32 = mybir.dt.float32
    ga = nc.dram_tensor("ga", [TBL_ROWS, 2 * D], bf16, kind="ExternalInput")
    gb = nc.dram_tensor("gb", [TBL_ROWS, 2 * D], bf16, kind="ExternalInput")
    didx = nc.dram_tensor("didx", [T * P, IDX_COLS], mybir.dt.int16,
                          kind="ExternalInput")
    oneh = nc.dram_tensor("oneh", [T * P, GRP * P], bf16,
                          kind="ExternalInput")
    wtab = nc.dram_tensor("wtab", [T * GRP * P, 2 * D], bf16,
                          kind="ExternalInput")
    out = nc.dram_tensor("partial", [P, 1], fp32, kind="ExternalOutput")

    Alu = mybir.AluOpType
    X = mybir.AxisListType.X
    Sq = mybir.ActivationFunctionType.Square
    dtab = [ga, gb]           # dst table by dst-half bucket

    with tile.TileContext(nc) as tc:
        with (
            tc.tile_pool(name="idx", bufs=4) as ipool,
            tc.tile_pool(name="gath", bufs=4) as gpool,
            tc.tile_pool(name="win", bufs=3) as wpool,
            tc.tile_pool(name="oh", bufs=3) as opool,
            tc.tile_pool(name="ps", bufs=2, space="PSUM") as pspool,
            tc.tile_pool(name="scr", bufs=3) as spool,
            tc.tile_pool(name="stats", bufs=1) as stpool,
        ):
            partials = stpool.tile([P, T * NB], fp32, tag="partials")
            t = 0
            for b in range(2):
                for _ in range(tiles_per_bucket[b]):
                    ei = ipool.tile([P, IDX_COLS], mybir.dt.int16, tag="ei")
                    nc.sync.dma_start(out=ei[:],
                                      in_=didx[t * P:(t + 1) * P, :])

                    v = gpool.tile([P, GRP, 2 * D], bf16, tag="v")
                    nc.gpsimd.dma_gather(v[:], dtab[b][:], ei[:], TILE_E,
                                         TILE_E, 2 * D, single_packet=False)

                    # group g's src block rows: wtab[(t*GRP+g)*P + k] ->
                    # xw[k, g, :]
                    xw = wpool.tile([P, GRP, 2 * D], bf16, tag="xw")
                    base = t * GRP * P
                    win_ap = bass.AP(
                        tensor=wtab[:].tensor,
                        offset=base * 2 * D,
                        ap=[[2 * D, P], [P * 2 * D, GRP], [1, 2 * D]])
                    nc.sync.dma_start(out=xw[:], in_=win_ap)

                    oh = opool.tile([P, GRP, P], bf16, tag="oh")
                    nc.sync.dma_start(out=oh[:],
                                      in_=oneh[t * P:(t + 1) * P, :])

                    for nb in range(NB):
                        ups = pspool.tile([P, GPB, 2 * D], fp32, tag="ups")
                        for gg in range(GPB):
                            g = nb * GPB + gg
                            nc.tensor.matmul(ups[:, gg, :], oh[:, g, :],
                                             xw[:, g, :],
                                             start=True, stop=True)
                        vs = v[:, nb * GPB:(nb + 1) * GPB, :]

                        prod = spool.tile([P, GPB, D], bf16, tag="prod")
                        agree = spool.tile([P, GPB], fp32, tag="agree")
                        diff = spool.tile([P, GPB, D], bf16, tag="diff")
                        sq = spool.tile([P, GPB, D], bf16, tag="sq")
                        sqsum = spool.tile([P, GPB], fp32, tag="sqsum")
                        junk = spool.tile([P, GPB], fp32, tag="junk")

                        nc.vector.tensor_tensor(out=prod[:],
                                                in0=ups[:, :, 0:D],
                                                in1=vs[:, :, 0:D],
                                                op=Alu.mult)
                        nc.vector.tensor_reduce(out=agree[:], in_=prod[:],
                                                axis=X, op=Alu.add)
                        nc.vector.tensor_tensor(out=diff[:],
                                                in0=ups[:, :, D:2 * D],
                                                in1=vs[:, :, D:2 * D],
                                                op=Alu.subtract)
                        nc.scalar.activation(out=sq[:], in_=diff[:], func=Sq)
                        nc.vector.tensor_reduce(out=sqsum[:], in_=sq[:],
                                                axis=X, op=Alu.add)
                        k = t * NB + nb
                        nc.vector.scalar_tensor_tensor(
                            out=junk[:], in0=agree[:], scalar=1.0,
                            in1=sqsum[:], op0=Alu.subtract, op1=Alu.mult,
                            accum_out=partials[:, k:k + 1])
                    t += 1

            total = stpool.tile([P, 1], fp32, tag="total")
            nc.vector.tensor_reduce(out=total[:], in_=partials[:], axis=X,
                                    op=Alu.add)
            nc.sync.dma_start(out=out[:], in_=total[:])
    nc.compile()
    _cache[key] = nc
    return nc


def _wrap_idx(flat_idx):
    """[n_tiles, TILE_E] local ids -> [n_tiles*P, IDX_COLS] int16 blocks.
    Logical j -> [j % 16, j // 16], replicated on all 8 16-row groups."""
    nt = flat_idx.shape[0]
    j = np.arange(TILE_E)
    w = np.zeros((nt, 16, IDX_COLS), np.int16)
    w[:, j % 16, j // 16] = flat_idx.astype(np.int16)
    return np.ascontiguousarray(np.tile(w, (1, 8, 1))).reshape(nt * P, IDX_COLS)


def _layout_bucket(sb, db):
    """Sort one core-bucket's edges by src and lay out greedy groups of
    128 consecutive edges; each group's src block starts at its first
    edge's src (any alignment). 128 consecutive src-sorted edges span far
    fewer than 128 distinct nodes here, so k_local always fits.

    Returns (n_tiles, dst_slots [n_tiles*TILE_E], base_of_group
    [n_tiles*GRP] (-1 = pad group), k_local, g_global, m)."""
    order = np.argsort(sb, kind="stable")
    sb = sb[order]
    db = db[order]
    ne = len(sb)
    starts, bases = [], []
    i = 0
    while i < ne:
        starts.append(i)
        bases.append(int(sb[i]))
        i = min(i + P, int(np.searchsorted(sb, sb[i] + P, side="left")))
    n_groups = max(1, len(starts))
    n_tiles = -(-n_groups // GRP)
    base_of_group = np.full(n_tiles * GRP, -1, np.int64)
    g_global = np.zeros(ne, np.int64)
    m = np.zeros(ne, np.int64)
    if ne:
        base_of_group[:len(starts)] = bases
        ends = starts[1:] + [ne]
        cnt = np.array(ends) - np.array(starts)
        g_global = np.repeat(np.arange(len(starts)), cnt)
        m = np.arange(ne) - np.repeat(np.array(starts), cnt)
    k_local = sb - base_of_group[g_global] if ne else np.zeros(0, np.int64)
    dst_slots = np.full(n_tiles * TILE_E, PAD_ROW, np.int64)
    dst_slots[g_global * P + m] = db
    return n_tiles, dst_slots, base_of_group, k_local, g_global, m


def kernel(re_, ir_h, src, dst):
    re_ = np.asarray(re_, dtype=np.float32)
    ir_h = np.asarray(ir_h, dtype=np.float32)
    g = np.concatenate([re_, ir_h], axis=1).astype(ml_dtypes.bfloat16)
    ga = np.zeros((TBL_ROWS, 2 * D), ml_dtypes.bfloat16)
    gb = np.zeros((TBL_ROWS, 2 * D), ml_dtypes.bfloat16)
    ga[:HALF] = g[:HALF]
    gb[:HALF] = g[HALF:]
    # global src table for window extraction (overread pad at the end)
    gfull = np.zeros((N_NODES + P, 2 * D), ml_dtypes.bfloat16)
    gfull[:N_NODES] = g

    s = np.asarray(src).astype(np.int64)
    d = np.asarray(dst).astype(np.int64)
    e_total = s.shape[0]
    bucket = (d >= HALF).astype(np.int64)   # dst half only

    # per (core, bucket) layouts; src ids stay global
    layouts = [[None] * 2 for _ in range(N_CORES)]
    for b in range(2):
        mask = bucket == b
        sb_all = s[mask]
        db_all = d[mask] - (HALF if b == 1 else 0)
        n = sb_all.shape[0]
        for c in range(N_CORES):
            lo = (n * c) // N_CORES
            hi = (n * (c + 1)) // N_CORES
            layouts[c][b] = _layout_bucket(sb_all[lo:hi], db_all[lo:hi])

    # shared per-bucket tile counts (max over cores)
    tiles_per_bucket = tuple(
        max(layouts[c][b][0] for c in range(N_CORES)) for b in range(2))
    T = sum(tiles_per_bucket)

    in_maps = []
    for c in range(N_CORES):
        dst_flat = np.full((T, TILE_E), PAD_ROW, np.int64)
        oneh = np.zeros((T * P, GRP * P), ml_dtypes.bfloat16)
        blocks = np.full(T * GRP, -1, np.int64)
        t0 = 0
        for b in range(2):
            nt, dst_slots, bog, k_local, g_global, m = layouts[c][b]
            dst_flat[t0:t0 + nt] = dst_slots.reshape(nt, TILE_E)
            blocks[t0 * GRP:t0 * GRP + nt * GRP] = bog
            # one-hot entries for real edges
            tl = g_global // GRP + t0
            gi = g_global % GRP
            oneh[tl * P + k_local, gi * P + m] = 1
            t0 += tiles_per_bucket[b]
        # window table: rows for (tile, group) = the group's 128-row block
        wtab = np.zeros((T * GRP, P, 2 * D), ml_dtypes.bfloat16)
        real = blocks >= 0
        rows = blocks[real, None] + np.arange(P)[None, :]
        wtab[real] = gfull[rows]
        in_maps.append({"ga": ga, "gb": gb,
                        "didx": _wrap_idx(dst_flat),
                        "oneh": np.ascontiguousarray(oneh),
                        "wtab": np.ascontiguousarray(
                            wtab.reshape(T * GRP * P, 2 * D))})

    nc = _build_program(tiles_per_bucket)
    res = run_bass_kernel_spmd(nc, in_maps, core_ids=list(range(N_CORES)))
    tot = 0.0
    for r in res.results:
        tot += float(r["partial"].sum(dtype=np.float64))
    return np.float32(-tot / e_total)


# revision 33
# speedup vs baseline: 2.3987x; 1.0374x over previous
"""Trainium2 Bass kernel for nn_Ir_Consistency_Loss (gnn_message_passing).

loss = mean_e (1 - re[src_e].re[dst_e]) * ||ir_h[src_e] - ir_h[dst_e]||^2

One-sided-gather design, edge-parallel across 8 NeuronCores.

The kernel's bottleneck is the Q7/Pool dma_gather descriptor path
(~8 ns per gathered row, engine-serial), so only the DST side is
gathered per edge; the SRC side is replicated on the PE:

  - Node features G = [re|ir] cast to bf16, split into half tables GA/GB
    (+zero pad rows) so dst-gather local row ids fit dma_gather's int16.
  - Edges bucketed by (src-half, dst-half), sharded across cores, then
    sorted by src. A group = 128 consecutive edges whose srcs lie in one
    128-node block; a tile = 32 groups = 4096 edges.
  - SRC side: the host pre-extracts each group's 128-row src block into a
    streamed tensor wtab [T*32*128, 256] and builds a one-hot
    [128 nodes x 128 edges] per group; a PE matmul (onehot^T @ block)
    replicates src rows into PSUM edge-major tiles. No per-edge gather.
  - DST side: one 4096-idx dma_gather per tile (PREPARE_ONLY + trigger,
    512B bf16 rows). Slot map j -> (j%128, j//128) makes group g line up
    exactly with v[:, g, :].
  - Pad edges have an all-zero one-hot column (u=0) and dst=zero row
    (v=0), so their contribution is exactly (1-0)*0 = 0 in any bucket.
  - DVE/ACT per 8-group PSUM batch: prod/reduce -> agree, diff, ACT
    square, reduce -> sqsum, then (agree-1)*sqsum accumulated into
    per-tile partials (negated loss).
  - Host: loss = -(sum of partials) / E.
"""

import numpy as np
import ml_dtypes

import concourse.bacc as bacc
import concourse.bass as bass
import concourse.mybir as mybir
import concourse.tile as tile
from concourse.bass_utils import run_bass_kernel_spmd

N_NODES = 50000
HALF = 25000
D = 128
N_CORES = 8
P = 128
GRP = 32                   # groups (of 128 edges) per tile
TILE_E = P * GRP           # 4096 edges per tile
IDX_COLS = TILE_E // 16    # int16 idx columns (wrap-16 layout)
PAD_ROW = HALF             # local id of an all-zero row in each table
TBL_ROWS = HALF + P        # half-table rows (zero padded; last block fits)

IDX_BUFS = 3
NB = 4                     # PSUM batches per tile
GPB = GRP // NB            # groups per batch (8)

_cache = {}


def _build_program(tiles_per_bucket):
    """tiles_per_bucket: per bucket (n_full_tiles, last_tile_groups)."""
    key = tuple(tiles_per_bucket)
    if key in _cache:
        return _cache[key]
    T = sum(nf + (1 if lg else 0) for nf, lg in tiles_per_bucket)
    nc = bacc.Bacc("TRN2", target_bir_lowering=False, debug=False,
                   num_devices=N_CORES)
    bf16 = mybir.dt.bfloat16
    fp32 = mybir.dt.float32
    ga = nc.dram_tensor("ga", [TBL_ROWS, 2 * D], bf16, kind="ExternalInput")
    gb = nc.dram_tensor("gb", [TBL_ROWS, 2 * D], bf16, kind="ExternalInput")
    didx = nc.dram_tensor("didx", [T * P, IDX_COLS], mybir.dt.int16,
                          kind="ExternalInput")
    oneh = nc.dram_tensor("oneh", [T * P, GRP * P], bf16,
                          kind="ExternalInput")
    wtab = nc.dram_tensor("wtab", [T * GRP * P, 2 * D], bf16,
                          kind="ExternalInput")
    out = nc.dram_tensor("partial", [P, 1], fp32, kind="ExternalOutput")

    Alu = mybir.AluOpType
    X = mybir.AxisListType.X
    Sq = mybir.ActivationFunctionType.Square
    dtab = [ga, gb]           # dst table by dst-half bucket

    with tile.TileContext(nc) as tc:
        with (
            tc.tile_pool(name="idx", bufs=4) as ipool,
            tc.tile_pool(name="gath", bufs=4) as gpool,
            tc.tile_pool(name="win", bufs=3) as wpool,
            tc.tile_pool(name="oh", bufs=3) as opool,
            tc.tile_pool(name="ps", bufs=2, space="PSUM") as pspool,
            tc.tile_pool(name="scr", bufs=3) as spool,
            tc.tile_pool(name="stats", bufs=1) as stpool,
        ):
            b_tot = sum(nf * NB + lg // GPB for nf, lg in tiles_per_bucket)
            partials = stpool.tile([P, b_tot], fp32, tag="partials")
            t = 0
            k = 0

            def emit_tile(t, k, b, ng):
                ei = ipool.tile([P, IDX_COLS], mybir.dt.int16, tag="ei")
                nc.sync.dma_start(out=ei[:],
                                  in_=didx[t * P:(t + 1) * P, :])

                ne = ng * P
                v = gpool.tile([P, ng, 2 * D], bf16, tag="v")
                nc.gpsimd.dma_gather(v[:], dtab[b][:],
                                     ei[:, 0:ne // 16], ne,
                                     ne, 2 * D, single_packet=False)

                # group g's src block rows: wtab[(t*GRP+g)*P + k] ->
                # xw[k, g, :]
                xw = wpool.tile([P, ng, 2 * D], bf16, tag="xw")
                base = t * GRP * P
                win_ap = bass.AP(
                    tensor=wtab[:].tensor,
                    offset=base * 2 * D,
                    ap=[[2 * D, P], [P * 2 * D, ng], [1, 2 * D]])
                nc.sync.dma_start(out=xw[:], in_=win_ap)

                oh = opool.tile([P, ng, P], bf16, tag="oh")
                nc.sync.dma_start(out=oh[:],
                                  in_=oneh[t * P:(t + 1) * P, 0:ng * P])

                for nb in range(ng // GPB):
                    ups = pspool.tile([P, GPB, 2 * D], fp32, tag="ups")
                    for gg in range(GPB):
                        g = nb * GPB + gg
                        nc.tensor.matmul(ups[:, gg, :], oh[:, g, :],
                                         xw[:, g, :],
                                         start=True, stop=True)
                    vs = v[:, nb * GPB:(nb + 1) * GPB, :]

                    prod = spool.tile([P, GPB, D], bf16, tag="prod")
                    agree = spool.tile([P, GPB], fp32, tag="agree")
                    diff = spool.tile([P, GPB, D], bf16, tag="diff")
                    sq = spool.tile([P, GPB, D], bf16, tag="sq")
                    sqsum = spool.tile([P, GPB], fp32, tag="sqsum")
                    junk = spool.tile([P, GPB], fp32, tag="junk")

                    nc.vector.tensor_tensor(out=prod[:],
                                            in0=ups[:, :, 0:D],
                                            in1=vs[:, :, 0:D],
                                            op=Alu.mult)
                    nc.vector.tensor_reduce(out=agree[:], in_=prod[:],
                                            axis=X, op=Alu.add)
                    nc.vector.tensor_tensor(out=diff[:],
                                            in0=ups[:, :, D:2 * D],
                                            in1=vs[:, :, D:2 * D],
                                            op=Alu.subtract)
                    nc.scalar.activation(out=sq[:], in_=diff[:], func=Sq)
                    nc.vector.tensor_reduce(out=sqsum[:], in_=sq[:],
                                            axis=X, op=Alu.add)
                    nc.vector.scalar_tensor_tensor(
                        out=junk[:], in0=agree[:], scalar=1.0,
                        in1=sqsum[:], op0=Alu.subtract, op1=Alu.mult,
                        accum_out=partials[:, k + nb:k + nb + 1])
                return ng // GPB

            for b in range(2):
                nf, lg = tiles_per_bucket[b]
                for _ in range(nf):
                    k += emit_tile(t, k, b, GRP)
                    t += 1
                if lg:
                    k += emit_tile(t, k, b, lg)
                    t += 1

            total = stpool.tile([P, 1], fp32, tag="total")
            nc.vector.tensor_reduce(out=total[:], in_=partials[:], axis=X,
                                    op=Alu.add)
            nc.sync.dma_start(out=out[:], in_=total[:])
    nc.compile()
    _cache[key] = nc
    return nc


def _wrap_idx(flat_idx):
    """[n_tiles, TILE_E] local ids -> [n_tiles*P, IDX_COLS] int16 blocks.
    Logical j -> [j % 16, j // 16], replicated on all 8 16-row groups."""
    nt = flat_idx.shape[0]
    j = np.arange(TILE_E)
    w = np.zeros((nt, 16, IDX_COLS), np.int16)
    w[:, j % 16, j // 16] = flat_idx.astype(np.int16)
    return np.ascontiguousarray(np.tile(w, (1, 8, 1))).reshape(nt * P, IDX_COLS)


def _layout_bucket(sb, db):
    """Sort one core-bucket's edges by src and lay out greedy groups of
    128 consecutive edges; each group's src block starts at its first
    edge's src (any alignment). 128 consecutive src-sorted edges span far
    fewer than 128 distinct nodes here, so k_local always fits.

    Returns (n_tiles, dst_slots [n_tiles*TILE_E], base_of_group
    [n_tiles*GRP] (-1 = pad group), k_local, g_global, m)."""
    order = np.argsort(sb, kind="stable")
    sb = sb[order]
    db = db[order]
    ne = len(sb)
    starts, bases = [], []
    i = 0
    while i < ne:
        starts.append(i)
        bases.append(int(sb[i]))
        i = min(i + P, int(np.searchsorted(sb, sb[i] + P, side="left")))
    n_groups = max(1, len(starts))
    bases_arr = np.zeros(n_groups, np.int64)
    g_global = np.zeros(ne, np.int64)
    m = np.zeros(ne, np.int64)
    if ne:
        bases_arr[:len(starts)] = bases
        ends = starts[1:] + [ne]
        cnt = np.array(ends) - np.array(starts)
        g_global = np.repeat(np.arange(len(starts)), cnt)
        m = np.arange(ne) - np.repeat(np.array(starts), cnt)
    k_local = sb - bases_arr[g_global] if ne else np.zeros(0, np.int64)
    return n_groups, bases_arr, k_local, g_global, m, db


def kernel(re_, ir_h, src, dst):
    re_ = np.asarray(re_, dtype=np.float32)
    ir_h = np.asarray(ir_h, dtype=np.float32)
    g = np.concatenate([re_, ir_h], axis=1).astype(ml_dtypes.bfloat16)
    ga = np.zeros((TBL_ROWS, 2 * D), ml_dtypes.bfloat16)
    gb = np.zeros((TBL_ROWS, 2 * D), ml_dtypes.bfloat16)
    ga[:HALF] = g[:HALF]
    gb[:HALF] = g[HALF:]
    # global src table for window extraction (overread pad at the end)
    gfull = np.zeros((N_NODES + P, 2 * D), ml_dtypes.bfloat16)
    gfull[:N_NODES] = g

    s = np.asarray(src).astype(np.int64)
    d = np.asarray(dst).astype(np.int64)
    e_total = s.shape[0]
    bucket = (d >= HALF).astype(np.int64)   # dst half only

    # per (core, bucket) layouts; src ids stay global
    layouts = [[None] * 2 for _ in range(N_CORES)]
    for b in range(2):
        mask = bucket == b
        sb_all = s[mask]
        db_all = d[mask] - (HALF if b == 1 else 0)
        n = sb_all.shape[0]
        for c in range(N_CORES):
            lo = (n * c) // N_CORES
            hi = (n * (c + 1)) // N_CORES
            layouts[c][b] = _layout_bucket(sb_all[lo:hi], db_all[lo:hi])

    # shared per-bucket tile shape (max group count over cores); the last
    # tile gathers only ceil(rem/GPB)*GPB groups instead of a full GRP
    tiles_per_bucket = []
    for b in range(2):
        gmax = max(layouts[c][b][0] for c in range(N_CORES))
        nf, rem = gmax // GRP, gmax % GRP
        lg = -(-rem // GPB) * GPB
        if lg == GRP:
            nf, lg = nf + 1, 0
        tiles_per_bucket.append((nf, lg))
    tiles_per_bucket = tuple(tiles_per_bucket)
    T = sum(nf + (1 if lg else 0) for nf, lg in tiles_per_bucket)

    in_maps = []
    for c in range(N_CORES):
        dst_flat = np.full((T, TILE_E), PAD_ROW, np.int64)
        oneh = np.zeros((T * P, GRP * P), ml_dtypes.bfloat16)
        blocks = np.full(T * GRP, -1, np.int64)
        t0 = 0
        for b in range(2):
            ng_c, bases_arr, k_local, g_global, m, db = layouts[c][b]
            nf, lg = tiles_per_bucket[b]
            nt = nf + (1 if lg else 0)
            # group g -> (tile row, slot within tile)
            gidx = np.arange(ng_c)
            tl_g = np.minimum(gidx // GRP, nf)
            gi_g = gidx - tl_g * GRP
            blocks[(t0 + tl_g) * GRP + gi_g] = bases_arr
            tl = t0 + tl_g[g_global]
            gi = gi_g[g_global]
            dst_flat[tl, gi * P + m] = db
            oneh[tl * P + k_local, gi * P + m] = 1
            t0 += nt
        # window table: rows for (tile, group) = the group's 128-row block
        wtab = np.zeros((T * GRP, P, 2 * D), ml_dtypes.bfloat16)
        real = blocks >= 0
        rows = blocks[real, None] + np.arange(P)[None, :]
        wtab[real] = gfull[rows]
        in_maps.append({"ga": ga, "gb": gb,
                        "didx": _wrap_idx(dst_flat),
                        "oneh": np.ascontiguousarray(oneh),
                        "wtab": np.ascontiguousarray(
                            wtab.reshape(T * GRP * P, 2 * D))})

    nc = _build_program(tiles_per_bucket)
    res = run_bass_kernel_spmd(nc, in_maps, core_ids=list(range(N_CORES)))
    tot = 0.0
    for r in res.results:
        tot += float(r["partial"].sum(dtype=np.float64))
    return np.float32(-tot / e_total)
